# revision 31
# baseline (speedup 1.0000x reference)
"""AltAttention (B=2,S=2048,D=1024,H=16, ALiBi + key-mask) on 8 TRN2 cores.

Sharding: core c = (b = c//4, head-group g = c%4 -> heads {g, g+4, g+8, g+12}).
Each core computes QKV for its 4 heads, attention, and a partial output
projection (row-split Wproj).  Host sums the 4 partials per batch and adds
bproj + Wproj @ bv (v-bias passes through softmax into a constant).

On-chip layout fully transposed: scores S^T=[k,q], context ctx^T=[dh,q],
output out^T=[dout,q].  All matmuls bf16 with fp32 PSUM.

v3 structure:
 - banding at (kt, half-window[512]) granularity, tiles with
   exp(-slope|k-q|) < e^-15 skipped; kt tiles processed in PAIRS sharing
   one [128,1024] PSUM tile and one ACT exp instruction.
 - heads 0,1 (steep): P = exp(S)*E (E table per diagonal offset).
   heads 2,3 (shallow): exp(-sl|k-q|) = [u(k)*v(q)]*R, v(q) cancels in
   softmax, u(k) folded into scaled V copies (vstR), R-table multiply only
   for tiles touching k>q.  E/R multiplies alternate DVE / GpSimd.
 - softmax: rowsums accumulated as a 65th PV output row;
   reciprocal_approx_fast + gpsimd partition_broadcast + one DVE multiply,
   per (head, half-window) right after its PV chain (fast ctx release).
 - q/k biases folded into PSUM evacuation (tensor_scalar add).
 - input x loaded over 4 DMA queues; phase-B tiles that only need window 0
   emitted between the two phase-A windows.
 - output partials in bf16, summed in fp32 on host.
"""

import sys

for _p in ("/opt/trn_rl_repo", "/opt/pypackages"):
    if _p not in sys.path:
        sys.path.insert(0, _p)

import numpy as np
import ml_dtypes

import concourse.bass as bass
from concourse import bacc
import concourse.mybir as mybir
import concourse.tile as tile
from concourse.bass_utils import run_bass_kernel_spmd

BF16 = ml_dtypes.bfloat16

B, S, D, H = 2, 2048, 1024, 16
DH = D // H
HPC = 4
SCALE = D ** -0.5
NKT = S // 128       # 16
NHF = S // 512       # 4 half-windows
NDT = D // 128       # 8
CENT = 1024

CUTS = [24, 96, 384, 99999]


def _band(hl, hf):
    cut = CUTS[hl]
    q0, q1 = hf * 512, hf * 512 + 512
    return [kt for kt in range(NKT)
            if kt * 128 < q1 + cut and (kt + 1) * 128 > q0 - cut]


BANDS = [[_band(hl, hf) for hf in range(NHF)] for hl in range(4)]


def _needs_e(hl, kt, hf):
    dlt = kt * 128 - hf * 512
    return hl < 2 or dlt > -128


EDELT = {}
for hl in range(4):
    ds = set()
    for hf in range(NHF):
        for kt in BANDS[hl][hf]:
            if _needs_e(hl, kt, hf):
                ds.add(kt * 128 - hf * 512)
    EDELT[hl] = sorted(ds)
EIDX = {hl: {d: i for i, d in enumerate(EDELT[hl])} for hl in range(4)}
ESLOT = [len(EDELT[hl]) for hl in range(4)]
EOFF = [0, ESLOT[0], ESLOT[0] + ESLOT[1], ESLOT[0] + ESLOT[1] + ESLOT[2]]
ETOT = sum(ESLOT)

_F32 = mybir.dt.float32
_BF = mybir.dt.bfloat16

Exp = mybir.ActivationFunctionType.Exp


def build_bass():
    nc = bacc.Bacc(None, target_bir_lowering=False)
    xt = nc.declare_dram_parameter("xt", [D, S], _BF, isOutput=False)
    wqk = nc.declare_dram_parameter("wqk", [D, 2 * HPC * DH], _BF, isOutput=False)
    qkb = nc.declare_dram_parameter("qkb", [128, 4], _F32, isOutput=False)
    wv = nc.declare_dram_parameter("wv", [D, HPC * DH], _BF, isOutput=False)
    wp = nc.declare_dram_parameter("wp", [HPC * DH, D], _BF, isOutput=False)
    etab = nc.declare_dram_parameter("etab", [128, ETOT * 512], _BF, isOutput=False)
    utab = nc.declare_dram_parameter("utab", [2 * S, 1], _F32, isOutput=False)
    mk = nc.declare_dram_parameter("mk", [S, 1], _F32, isOutput=False)
    out = nc.declare_dram_parameter("out", [D, S], _BF, isOutput=True)

    with tile.TileContext(nc) as tc:
        with (
            tc.tile_pool(name="consts", bufs=1) as consts,
            tc.tile_pool(name="wqk_p", bufs=1) as wqk_p,
            tc.tile_pool(name="wv_p", bufs=1) as wv_p,
            tc.tile_pool(name="kqt_p", bufs=1) as kqt_p,
            tc.tile_pool(name="vst_p", bufs=1) as vst_p,
            tc.tile_pool(name="xt_p", bufs=16) as xt_p,
            tc.tile_pool(name="ear_p", bufs=1) as ear_p,
            tc.tile_pool(name="p_p", bufs=6) as p_p,
            tc.tile_pool(name="wp_p", bufs=1) as wp_p,
            tc.tile_pool(name="ot_p", bufs=2) as ot_p,
            tc.tile_pool(name="sm_p", bufs=3) as sm_p,
            tc.tile_pool(name="ps", bufs=3, space="PSUM") as ps,
            tc.tile_pool(name="psc", bufs=2, space="PSUM") as psc,
        ):
            # ---- phase-A loads spread over 4 DMA queues ----
            xts_w = [[None] * NDT for _ in range(2)]

            def load_xt(stp, dt, eng, split=False):
                t = xt_p.tile([128, 1024], _BF, tag="xt", name=f"xt{stp}_{dt}")
                if split:
                    for cc in (0, 512):
                        eng.dma_start(
                            out=t[:, cc:cc + 512],
                            in_=xt[dt * 128:(dt + 1) * 128,
                                   stp * 1024 + cc:stp * 1024 + cc + 512])
                else:
                    eng.dma_start(
                        out=t, in_=xt[dt * 128:(dt + 1) * 128,
                                      stp * 1024:(stp + 1) * 1024])
                xts_w[stp][dt] = t

            wqk_s = [None] * NDT

            def load_wqk(dt, eng):
                t = wqk_p.tile([128, 512], _BF, tag=f"wqk{dt}", name=f"wqk{dt}")
                eng.dma_start(out=t, in_=wqk[dt * 128:(dt + 1) * 128, :])
                wqk_s[dt] = t

            qmap = {0: nc.sync, 1: nc.scalar, 2: nc.gpsimd}
            for dt in range(NDT):
                load_wqk(dt, qmap[dt % 3])
            # window-0 x in exact consumption order: first halves then second
            xt_half = {}

            def load_xt0_half(dt, cc, eng):
                if dt not in xt_half:
                    xt_half[dt] = xt_p.tile([128, 1024], _BF, tag="xt",
                                            name=f"xt0_{dt}")
                    xts_w[0][dt] = xt_half[dt]
                eng.dma_start(out=xt_half[dt][:, cc:cc + 512],
                              in_=xt[dt * 128:(dt + 1) * 128, cc:cc + 512])

            for cc in (0, 512):
                for dt in range(NDT):
                    load_xt0_half(dt, cc, qmap[dt % 3])
            wv_s = []
            for dt in range(NDT):
                t = wv_p.tile([128, 256], _BF, tag=f"wv{dt}", name=f"wv{dt}")
                nc.gpsimd.dma_start(out=t, in_=wv[dt * 128:(dt + 1) * 128, :])
                wv_s.append(t)
            for dt in range(NDT):
                load_xt(1, dt, qmap[dt % 3])

            # ---- ACT exp table warm-up ----
            dum = consts.tile([1, 1], _F32)
            nc.vector.memset(dum, 0.0)
            nc.scalar.activation(dum, dum, Exp)

            qkb_s = consts.tile([128, 4], _F32)
            nc.gpsimd.dma_start(out=qkb_s, in_=qkb[:, :])
            mk_s = consts.tile([128, NKT], _F32)
            nc.gpsimd.dma_start(
                out=mk_s, in_=mk.rearrange("(f p) a -> p (f a)", p=128))
            utab_s = consts.tile([128, 2 * NKT], _F32)
            nc.gpsimd.dma_start(
                out=utab_s, in_=utab.rearrange("(j f p) a -> p (j f a)",
                                               j=2, p=128))
            wp_s = []
            for hp in range(2):
                t = wp_p.tile([128, D], _BF, tag=f"wp{hp}", name=f"wp{hp}")
                nc.scalar.dma_start(out=t, in_=wp[hp * 128:(hp + 1) * 128, :])
                wp_s.append(t)
            # E tables: h0+h1 slots first (needed early), rest behind
            earena = ear_p.tile([128, ETOT * 512], _BF)
            c01 = EOFF[2] * 512  # columns for heads 0,1
            nc.sync.dma_start(out=earena[:, 0:c01], in_=etab[:, 0:c01])
            rest = ETOT * 512 - c01
            nch = 4
            w_ = rest // nch
            for c4 in range(nch):
                lo = c01 + c4 * w_
                hi = c01 + (c4 + 1) * w_ + (rest - nch * w_ if c4 == nch - 1 else 0)
                nc.sync.dma_start(out=earena[:, lo:hi], in_=etab[:, lo:hi])

            # ---- persistent activation tensors ----
            qq = [kqt_p.tile([128, S], _BF, tag=f"qq{p}", name=f"qq{p}")
                  for p in range(2)]
            kk = [kqt_p.tile([128, S], _BF, tag=f"kk{p}", name=f"kk{p}")
                  for p in range(2)]
            vst = [vst_p.tile([128, HPC * 65], _BF, tag=f"vst{kt}", name=f"vst{kt}")
                   for kt in range(NKT)]
            vstR = [vst_p.tile([128, 2 * 65], _BF, tag=f"vstR{kt}", name=f"vstR{kt}")
                    for kt in range(NKT)]
            ctx_s = [kqt_p.tile([128, S], _BF, tag=f"cs{hp}", name=f"cs{hp}")
                     for hp in range(2)]

            for kt in range(NKT):
                for h in range(HPC):
                    nc.vector.memset(vst[kt][:, h * 65 + 64:h * 65 + 65], 1.0)

            # ================= phase A (one window) =================
            def emit_A(stp):
                xts = xts_w[stp]
                for rt in range(HPC):
                    qk_ps = ps.tile([128, 1024], _F32, tag="ps2", name="qk_ps")
                    for c0 in (0, 512):
                        for dt in range(NDT):
                            nc.tensor.matmul(
                                qk_ps[:, c0:c0 + 512],
                                lhsT=wqk_s[dt][:, rt * 128:(rt + 1) * 128],
                                rhs=xts[dt][:, c0:c0 + 512],
                                start=(dt == 0), stop=(dt == NDT - 1),
                            )
                    dst = (qq if rt % 2 == 0 else kk)[rt // 2]
                    nc.vector.tensor_scalar_add(
                        dst[:, stp * 1024:(stp + 1) * 1024], qk_ps,
                        qkb_s[:, rt:rt + 1])
                for sg in range(2):
                    v_ps = ps.tile([128, 1024], _F32, tag="ps2", name="v_ps")
                    for s4 in range(4):
                        sub = sg * 4 + s4
                        for dt in range(NDT):
                            nc.tensor.matmul(
                                v_ps[:, s4 * 256:s4 * 256 + 256],
                                lhsT=xts[dt][:, sub * 128:(sub + 1) * 128],
                                rhs=wv_s[dt],
                                start=(dt == 0), stop=(dt == NDT - 1),
                            )
                    for s4 in range(4):
                        kt_i = stp * 8 + sg * 4 + s4
                        for h in range(HPC):
                            nc.vector.tensor_copy(
                                out=vst[kt_i][:, h * 65:h * 65 + 64],
                                in_=v_ps[:, s4 * 256 + h * 64:s4 * 256 + (h + 1) * 64])
                        nc.vector.tensor_scalar_mul(
                            vst[kt_i][:, :], vst[kt_i][:, :],
                            mk_s[:, kt_i:kt_i + 1])
                        for j in range(2):
                            nc.vector.tensor_scalar_mul(
                                vstR[kt_i][:, j * 65:(j + 1) * 65],
                                vst[kt_i][:, (2 + j) * 65:(3 + j) * 65],
                                utab_s[:, j * NKT + kt_i:j * NKT + kt_i + 1])

            # ================= phase B (software-pipelined) =================
            # tensor engine is strictly in-order: a PV matmul waiting on
            # exp+E of its own tile would stall the next score matmul.  So
            # PV groups and norms are deferred through a small lag queue and
            # emitted behind the next pairs' score/exp fronts.
            emul_tog = [0]
            pend = []
            LAG = 3

            def drain(keep):
                while len(pend) > keep:
                    pend.pop(0)()

            def emit_hf(h, hf):
                """score+exp+E+PV and softmax-normalize (head h, half-window hf)."""
                hp, half = h // 2, h % 2
                lo, hi = half * 64, half * 64 + 64
                hr = half * 64
                kts = BANDS[h][hf]
                ctx_ps = psc.tile([65, 512], _F32, tag="ctx", name="ctx_ps")
                qs = qq[hp][lo:hi, hf * 512:hf * 512 + 512]
                pairs = [kts[i:i + 2] for i in range(0, len(kts), 2)]
                npairs = len(pairs)
                for pi, pr in enumerate(pairs):
                    s2 = ps.tile([128, 1024], _F32, tag="ps2", name="s2")
                    for j, kt in enumerate(pr):
                        nc.tensor.matmul(
                            s2[:, j * 512:j * 512 + 512],
                            lhsT=kk[hp][lo:hi, kt * 128:(kt + 1) * 128],
                            rhs=qs, start=True, stop=True,
                        )
                    wd = len(pr) * 512
                    p2 = p_p.tile([128, 1024], _BF, tag="p", name="p2")
                    nc.scalar.activation(p2[:, 0:wd], s2[:, 0:wd], Exp)
                    for j, kt in enumerate(pr):
                        if _needs_e(h, kt, hf):
                            ei = EOFF[h] + EIDX[h][kt * 128 - hf * 512]
                            nc.vector.tensor_mul(
                                p2[:, j * 512:j * 512 + 512],
                                p2[:, j * 512:j * 512 + 512],
                                earena[:, ei * 512:(ei + 1) * 512])

                    def pv_group(pr=pr, p2=p2, pi=pi):
                        for j, kt in enumerate(pr):
                            if h < 2:
                                lhsT = vst[kt][:, h * 65:(h + 1) * 65]
                            else:
                                lhsT = vstR[kt][:, (h - 2) * 65:(h - 1) * 65]
                            nc.tensor.matmul(
                                ctx_ps, lhsT=lhsT,
                                rhs=p2[:, j * 512:j * 512 + 512],
                                start=(pi == 0 and j == 0),
                                stop=(pi == npairs - 1 and j == len(pr) - 1),
                                skip_group_check=True,
                            )
                    pend.append(pv_group)
                    drain(LAG)

                def norm():
                    rr = sm_p.tile([1, 512], _F32, tag="rr", name="rr")
                    nc.scalar.copy(out=rr, in_=ctx_ps[64:65, :])
                    r_s = sm_p.tile([1, 512], _F32, tag="r", name="r_s")
                    nc.vector.reciprocal_approx_fast(out=r_s, in_=rr)
                    rb = sm_p.tile([64, 512], _F32, tag="rb", name="rb")
                    nc.gpsimd.partition_broadcast(rb, r_s)
                    nc.vector.tensor_mul(
                        ctx_s[hp][hr:hr + 64, hf * 512:hf * 512 + 512],
                        ctx_ps[0:64, :], rb)
                pend.append(norm)
                drain(LAG)

            def emit_proj(q0, width):
                for dt in range(NDT):
                    o_ps = ps.tile([128, 1024], _F32, tag="ps2", name="o_ps")
                    for cc in range(0, width, 512):
                        for hp2 in range(2):
                            nc.tensor.matmul(
                                o_ps[:, cc:cc + 512],
                                lhsT=wp_s[hp2][:, dt * 128:(dt + 1) * 128],
                                rhs=ctx_s[hp2][:, q0 + cc:q0 + cc + 512],
                                start=(hp2 == 0), stop=(hp2 == 1),
                            )
                    o_s = ot_p.tile([128, 1024], _BF, tag="ot", name="o_s")
                    if dt % 2 == 0:
                        nc.scalar.copy(out=o_s[:, 0:width], in_=o_ps[:, 0:width])
                    else:
                        nc.vector.tensor_copy(out=o_s[:, 0:width],
                                              in_=o_ps[:, 0:width])
                    nc.sync.dma_start(
                        out=out[dt * 128:(dt + 1) * 128, q0:q0 + width],
                        in_=o_s[:, 0:width])

            # ================= schedule =================
            emit_A(0)
            emit_hf(0, 0)       # only needs window-0 data
            emit_hf(1, 0)
            emit_A(1)
            emit_hf(0, 1)
            emit_hf(1, 1)
            emit_hf(2, 0)
            emit_hf(2, 1)
            emit_hf(3, 0)
            emit_hf(3, 1)
            emit_hf(3, 2)
            emit_hf(2, 2)
            emit_proj(0, 1024)
            emit_hf(1, 2)
            emit_hf(0, 2)
            emit_hf(3, 3)
            emit_proj(1024, 512)
            emit_hf(2, 3)
            emit_hf(1, 3)
            emit_hf(0, 3)
            drain(0)
            emit_proj(1536, 512)
    nc.finalize()
    return nc


_NC = None


def _get_nc():
    global _NC
    if _NC is None:
        _NC = build_bass()
    return _NC


def _host_inputs(inputs, mask, Wqkv, bqkv, Wproj, bproj):
    x = np.asarray(inputs, np.float32)
    mask = np.asarray(mask)
    Wqkv = np.asarray(Wqkv, np.float32)
    bqkv = np.asarray(bqkv, np.float32)
    Wproj = np.asarray(Wproj, np.float32)

    start = 2.0 ** (-8.0 / H)
    slopes = start ** np.arange(1, H + 1, dtype=np.float64)

    per_g = {}
    ii = np.arange(128, dtype=np.float64)[:, None]
    jj = np.arange(512, dtype=np.float64)[None, :]
    for g in range(4):
        heads = [g, g + 4, g + 8, g + 12]
        wqk_ = np.empty((D, 2 * HPC * DH), np.float32)
        qkb_ = np.empty((128, 4), np.float32)
        wv_ = np.empty((D, HPC * DH), np.float32)
        wp_ = np.empty((HPC * DH, D), np.float32)
        etab_ = np.zeros((128, ETOT * 512), BF16)
        utab_ = np.empty((2 * S, 1), np.float32)
        for hl, hh in enumerate(heads):
            r0 = hh * 3 * DH
            p, half = hl // 2, hl % 2
            qcol = (2 * p) * 128 + half * 64
            kcol = (2 * p + 1) * 128 + half * 64
            wqk_[:, qcol:qcol + 64] = Wqkv[r0:r0 + DH, :].T * SCALE
            wqk_[:, kcol:kcol + 64] = Wqkv[r0 + DH:r0 + 2 * DH, :].T
            qkb_[half * 64:(half + 1) * 64, 2 * p] = bqkv[r0:r0 + DH] * SCALE
            qkb_[half * 64:(half + 1) * 64, 2 * p + 1] = bqkv[r0 + DH:r0 + 2 * DH]
            wv_[:, hl * 64:(hl + 1) * 64] = Wqkv[r0 + 2 * DH:r0 + 3 * DH, :].T
            wp_[hl * 64:(hl + 1) * 64, :] = Wproj[:, hh * DH:(hh + 1) * DH].T
            sl = slopes[hh]
            for dlt in EDELT[hl]:
                ei = EOFF[hl] + EIDX[hl][dlt]
                dd = dlt + ii - jj                    # k - q
                if hl < 2:
                    blk = np.exp(-sl * np.abs(dd))
                else:
                    blk = np.where(dd <= 0, 1.0, np.exp(-2.0 * sl * dd))
                etab_[:, ei * 512:(ei + 1) * 512] = blk
            if hl >= 2:
                kkk = np.arange(S, dtype=np.float64)
                utab_[(hl - 2) * S:(hl - 1) * S, 0] = np.exp(sl * (kkk - CENT))
        per_g[g] = dict(wqk=wqk_.astype(BF16), qkb=qkb_,
                        wv=wv_.astype(BF16),
                        wp=wp_.astype(BF16), etab=etab_, utab=utab_)

    in_maps = []
    for c in range(8):
        b, g = c // 4, c % 4
        m = dict(per_g[g])
        m["xt"] = np.ascontiguousarray(x[b].T).astype(BF16)
        m["mk"] = mask[b].astype(np.float32).reshape(S, 1)
        in_maps.append(m)
    return in_maps


def kernel(inputs, mask, Wqkv, bqkv, Wproj, bproj, _want_trace=False):
    nc = _get_nc()
    in_maps = _host_inputs(inputs, mask, Wqkv, bqkv, Wproj, bproj)
    res = run_bass_kernel_spmd(nc, in_maps, core_ids=list(range(8)),
                               trace=_want_trace)
    outs = res.results
    out = np.zeros((B, S, D), np.float32)
    for c in range(8):
        out[c // 4] += np.asarray(outs[c]["out"], np.float32).T
    # v-bias flows through softmax (weights sum to 1) into a constant:
    bv = np.asarray(bqkv, np.float32).reshape(3 * H, DH)[2::3].reshape(D)
    out += np.asarray(Wproj, np.float32) @ bv + np.asarray(bproj, np.float32)
    if _want_trace:
        kernel.last_result = res
    return out


# revision 33
# speedup vs baseline: 1.0551x; 1.0551x over previous
"""AltAttention (B=2,S=2048,D=1024,H=16, ALiBi + key-mask) on 8 TRN2 cores.

Sharding: core c = (b = c//4, head-group g = c%4 -> heads {g, g+4, g+8, g+12}).
Each core computes QKV for its 4 heads, attention, and a partial output
projection (row-split Wproj).  Host sums the 4 partials per batch and adds
bproj + Wproj @ bv (v-bias passes through softmax into a constant).

On-chip layout fully transposed: scores S^T=[k,q], context ctx^T=[dh,q],
output out^T=[dout,q].  All matmuls bf16 with fp32 PSUM.

v3 structure:
 - banding at (kt, half-window[512]) granularity, tiles with
   exp(-slope|k-q|) < e^-15 skipped; kt tiles processed in PAIRS sharing
   one [128,1024] PSUM tile and one ACT exp instruction.
 - heads 0,1 (steep): P = exp(S)*E (E table per diagonal offset).
   heads 2,3 (shallow): exp(-sl|k-q|) = [u(k)*v(q)]*R, v(q) cancels in
   softmax, u(k) folded into scaled V copies (vstR), R-table multiply only
   for tiles touching k>q.  E/R multiplies alternate DVE / GpSimd.
 - softmax: rowsums accumulated as a 65th PV output row;
   reciprocal_approx_fast + gpsimd partition_broadcast + one DVE multiply,
   per (head, half-window) right after its PV chain (fast ctx release).
 - q/k biases folded into PSUM evacuation (tensor_scalar add).
 - input x loaded over 4 DMA queues; phase-B tiles that only need window 0
   emitted between the two phase-A windows.
 - output partials in bf16, summed in fp32 on host.
"""

import sys

for _p in ("/opt/trn_rl_repo", "/opt/pypackages"):
    if _p not in sys.path:
        sys.path.insert(0, _p)

import numpy as np
import ml_dtypes

import concourse.bass as bass
from concourse import bacc
import concourse.mybir as mybir
import concourse.tile as tile
from concourse.bass_utils import run_bass_kernel_spmd

BF16 = ml_dtypes.bfloat16

B, S, D, H = 2, 2048, 1024, 16
DH = D // H
HPC = 4
SCALE = D ** -0.5
NKT = S // 128       # 16
NHF = S // 512       # 4 half-windows
NDT = D // 128       # 8
CENT = 1024

CUTS = [24, 96, 384, 99999]


def _band(hl, hf):
    cut = CUTS[hl]
    q0, q1 = hf * 512, hf * 512 + 512
    return [kt for kt in range(NKT)
            if kt * 128 < q1 + cut and (kt + 1) * 128 > q0 - cut]


BANDS = [[_band(hl, hf) for hf in range(NHF)] for hl in range(4)]


def _needs_e(hl, kt, hf):
    dlt = kt * 128 - hf * 512
    return hl < 2 or dlt > -128


EDELT = {}
for hl in range(4):
    ds = set()
    for hf in range(NHF):
        for kt in BANDS[hl][hf]:
            if _needs_e(hl, kt, hf):
                ds.add(kt * 128 - hf * 512)
    EDELT[hl] = sorted(ds)
EIDX = {hl: {d: i for i, d in enumerate(EDELT[hl])} for hl in range(4)}
ESLOT = [len(EDELT[hl]) for hl in range(4)]
EOFF = [0, ESLOT[0], ESLOT[0] + ESLOT[1], ESLOT[0] + ESLOT[1] + ESLOT[2]]
ETOT = sum(ESLOT)

_F32 = mybir.dt.float32
_BF = mybir.dt.bfloat16

Exp = mybir.ActivationFunctionType.Exp


def build_bass():
    nc = bacc.Bacc(None, target_bir_lowering=False)
    xt = nc.declare_dram_parameter("xt", [D, S], _BF, isOutput=False)
    wqk = nc.declare_dram_parameter("wqk", [D, 2 * HPC * DH], _BF, isOutput=False)
    qkb = nc.declare_dram_parameter("qkb", [128, 4], _F32, isOutput=False)
    wv = nc.declare_dram_parameter("wv", [D, HPC * DH], _BF, isOutput=False)
    wp = nc.declare_dram_parameter("wp", [HPC * DH, D], _BF, isOutput=False)
    etab = nc.declare_dram_parameter("etab", [128, ETOT * 512], _BF, isOutput=False)
    utab = nc.declare_dram_parameter("utab", [2 * S, 1], _F32, isOutput=False)
    mk = nc.declare_dram_parameter("mk", [S, 1], _F32, isOutput=False)
    out = nc.declare_dram_parameter("out", [D, S], _BF, isOutput=True)

    with tile.TileContext(nc) as tc:
        with (
            tc.tile_pool(name="consts", bufs=1) as consts,
            tc.tile_pool(name="wqk_p", bufs=1) as wqk_p,
            tc.tile_pool(name="wv_p", bufs=1) as wv_p,
            tc.tile_pool(name="kqt_p", bufs=1) as kqt_p,
            tc.tile_pool(name="vst_p", bufs=1) as vst_p,
            tc.tile_pool(name="xt_p", bufs=16) as xt_p,
            tc.tile_pool(name="ear_p", bufs=1) as ear_p,
            tc.tile_pool(name="p_p", bufs=6) as p_p,
            tc.tile_pool(name="wp_p", bufs=1) as wp_p,
            tc.tile_pool(name="ot_p", bufs=2) as ot_p,
            tc.tile_pool(name="sm_p", bufs=3) as sm_p,
            tc.tile_pool(name="ps", bufs=3, space="PSUM") as ps,
            tc.tile_pool(name="psc", bufs=2, space="PSUM") as psc,
        ):
            # ---- phase-A loads spread over 4 DMA queues ----
            xts_w = [[None] * NDT for _ in range(2)]

            def load_xt(stp, dt, eng, split=False):
                t = xt_p.tile([128, 1024], _BF, tag="xt", name=f"xt{stp}_{dt}")
                if split:
                    for cc in (0, 512):
                        eng.dma_start(
                            out=t[:, cc:cc + 512],
                            in_=xt[dt * 128:(dt + 1) * 128,
                                   stp * 1024 + cc:stp * 1024 + cc + 512])
                else:
                    eng.dma_start(
                        out=t, in_=xt[dt * 128:(dt + 1) * 128,
                                      stp * 1024:(stp + 1) * 1024])
                xts_w[stp][dt] = t

            wqk_s = [None] * NDT

            def load_wqk(dt, eng):
                t = wqk_p.tile([128, 512], _BF, tag=f"wqk{dt}", name=f"wqk{dt}")
                eng.dma_start(out=t, in_=wqk[dt * 128:(dt + 1) * 128, :])
                wqk_s[dt] = t

            for dt in (0, 1, 2, 3):
                load_wqk(dt, nc.scalar)
            for dt in (4, 5, 6, 7):
                load_wqk(dt, nc.gpsimd)
            for dt in (0, 1, 2):
                load_xt(0, dt, nc.sync, split=True)
            for dt in (3, 4, 5):
                load_xt(0, dt, nc.scalar, split=True)
            for dt in (6, 7):
                load_xt(0, dt, nc.gpsimd, split=True)
            wv_s = []
            for dt in range(NDT):
                t = wv_p.tile([128, 256], _BF, tag=f"wv{dt}", name=f"wv{dt}")
                nc.gpsimd.dma_start(out=t, in_=wv[dt * 128:(dt + 1) * 128, :])
                wv_s.append(t)
            for dt in (0, 1, 2):
                load_xt(1, dt, nc.sync)
            for dt in (3, 4, 5):
                load_xt(1, dt, nc.scalar)
            for dt in (6, 7):
                load_xt(1, dt, nc.gpsimd)

            # ---- ACT exp table warm-up ----
            dum = consts.tile([1, 1], _F32)
            nc.vector.memset(dum, 0.0)
            nc.scalar.activation(dum, dum, Exp)

            qkb_s = consts.tile([128, 4], _F32)
            nc.gpsimd.dma_start(out=qkb_s, in_=qkb[:, :])
            mk_s = consts.tile([128, NKT], _F32)
            nc.gpsimd.dma_start(
                out=mk_s, in_=mk.rearrange("(f p) a -> p (f a)", p=128))
            utab_s = consts.tile([128, 2 * NKT], _F32)
            nc.gpsimd.dma_start(
                out=utab_s, in_=utab.rearrange("(j f p) a -> p (j f a)",
                                               j=2, p=128))
            wp_s = []
            for hp in range(2):
                t = wp_p.tile([128, D], _BF, tag=f"wp{hp}", name=f"wp{hp}")
                nc.scalar.dma_start(out=t, in_=wp[hp * 128:(hp + 1) * 128, :])
                wp_s.append(t)
            # E tables: h0+h1 slots first (needed early), rest behind
            earena = ear_p.tile([128, ETOT * 512], _BF)
            c01 = EOFF[2] * 512  # columns for heads 0,1
            nc.sync.dma_start(out=earena[:, 0:c01], in_=etab[:, 0:c01])
            rest = ETOT * 512 - c01
            nch = 4
            w_ = rest // nch
            for c4 in range(nch):
                lo = c01 + c4 * w_
                hi = c01 + (c4 + 1) * w_ + (rest - nch * w_ if c4 == nch - 1 else 0)
                nc.sync.dma_start(out=earena[:, lo:hi], in_=etab[:, lo:hi])

            # ---- persistent activation tensors ----
            qq = [kqt_p.tile([128, S], _BF, tag=f"qq{p}", name=f"qq{p}")
                  for p in range(2)]
            kk = [kqt_p.tile([128, S], _BF, tag=f"kk{p}", name=f"kk{p}")
                  for p in range(2)]
            vst = [vst_p.tile([128, HPC * 65], _BF, tag=f"vst{kt}", name=f"vst{kt}")
                   for kt in range(NKT)]
            vstR = [vst_p.tile([128, 2 * 65], _BF, tag=f"vstR{kt}", name=f"vstR{kt}")
                    for kt in range(NKT)]
            ctx_s = [kqt_p.tile([128, S], _BF, tag=f"cs{hp}", name=f"cs{hp}")
                     for hp in range(2)]

            for kt in range(NKT):
                for h in range(HPC):
                    nc.vector.memset(vst[kt][:, h * 65 + 64:h * 65 + 65], 1.0)

            # ================= phase A (one window) =================
            def emit_A(stp):
                xts = xts_w[stp]
                for rt in range(HPC):
                    qk_ps = ps.tile([128, 1024], _F32, tag="ps2", name="qk_ps")
                    for c0 in (0, 512):
                        for dt in range(NDT):
                            nc.tensor.matmul(
                                qk_ps[:, c0:c0 + 512],
                                lhsT=wqk_s[dt][:, rt * 128:(rt + 1) * 128],
                                rhs=xts[dt][:, c0:c0 + 512],
                                start=(dt == 0), stop=(dt == NDT - 1),
                            )
                    dst = (qq if rt % 2 == 0 else kk)[rt // 2]
                    nc.vector.tensor_scalar_add(
                        dst[:, stp * 1024:(stp + 1) * 1024], qk_ps,
                        qkb_s[:, rt:rt + 1])
                for sg in range(2):
                    v_ps = ps.tile([128, 1024], _F32, tag="ps2", name="v_ps")
                    for s4 in range(4):
                        sub = sg * 4 + s4
                        for dt in range(NDT):
                            nc.tensor.matmul(
                                v_ps[:, s4 * 256:s4 * 256 + 256],
                                lhsT=xts[dt][:, sub * 128:(sub + 1) * 128],
                                rhs=wv_s[dt],
                                start=(dt == 0), stop=(dt == NDT - 1),
                            )
                    for s4 in range(4):
                        kt_i = stp * 8 + sg * 4 + s4
                        for h in range(HPC):
                            nc.vector.tensor_copy(
                                out=vst[kt_i][:, h * 65:h * 65 + 64],
                                in_=v_ps[:, s4 * 256 + h * 64:s4 * 256 + (h + 1) * 64])
                        nc.vector.tensor_scalar_mul(
                            vst[kt_i][:, :], vst[kt_i][:, :],
                            mk_s[:, kt_i:kt_i + 1])
                        for j in range(2):
                            nc.vector.tensor_scalar_mul(
                                vstR[kt_i][:, j * 65:(j + 1) * 65],
                                vst[kt_i][:, (2 + j) * 65:(3 + j) * 65],
                                utab_s[:, j * NKT + kt_i:j * NKT + kt_i + 1])

            # ================= phase B (software-pipelined) =================
            # tensor engine is strictly in-order: a PV matmul waiting on
            # exp+E of its own tile would stall the next score matmul.  So
            # PV groups and norms are deferred through a small lag queue and
            # emitted behind the next pairs' score/exp fronts.
            emul_tog = [0]
            pend = []
            LAG = 4

            def drain(keep):
                while len(pend) > keep:
                    pend.pop(0)()

            def emit_hf(h, hf):
                """score+exp+E+PV and softmax-normalize (head h, half-window hf)."""
                hp, half = h // 2, h % 2
                lo, hi = half * 64, half * 64 + 64
                hr = half * 64
                kts = BANDS[h][hf]
                ctx_ps = psc.tile([65, 512], _F32, tag="ctx", name="ctx_ps")
                qs = qq[hp][lo:hi, hf * 512:hf * 512 + 512]
                pairs = [kts[i:i + 2] for i in range(0, len(kts), 2)]
                npairs = len(pairs)
                for pi, pr in enumerate(pairs):
                    s2 = ps.tile([128, 1024], _F32, tag="ps2", name="s2")
                    for j, kt in enumerate(pr):
                        nc.tensor.matmul(
                            s2[:, j * 512:j * 512 + 512],
                            lhsT=kk[hp][lo:hi, kt * 128:(kt + 1) * 128],
                            rhs=qs, start=True, stop=True,
                        )
                    wd = len(pr) * 512
                    p2 = p_p.tile([128, 1024], _BF, tag="p", name="p2")
                    nc.scalar.activation(p2[:, 0:wd], s2[:, 0:wd], Exp)
                    for j, kt in enumerate(pr):
                        if _needs_e(h, kt, hf):
                            ei = EOFF[h] + EIDX[h][kt * 128 - hf * 512]
                            nc.vector.tensor_mul(
                                p2[:, j * 512:j * 512 + 512],
                                p2[:, j * 512:j * 512 + 512],
                                earena[:, ei * 512:(ei + 1) * 512])

                    def pv_group(pr=pr, p2=p2, pi=pi):
                        for j, kt in enumerate(pr):
                            if h < 2:
                                lhsT = vst[kt][:, h * 65:(h + 1) * 65]
                            else:
                                lhsT = vstR[kt][:, (h - 2) * 65:(h - 1) * 65]
                            nc.tensor.matmul(
                                ctx_ps, lhsT=lhsT,
                                rhs=p2[:, j * 512:j * 512 + 512],
                                start=(pi == 0 and j == 0),
                                stop=(pi == npairs - 1 and j == len(pr) - 1),
                                skip_group_check=True,
                            )
                    pend.append(pv_group)
                    drain(LAG)

                def norm():
                    rr = sm_p.tile([1, 512], _F32, tag="rr", name="rr")
                    nc.scalar.copy(out=rr, in_=ctx_ps[64:65, :])
                    r_s = sm_p.tile([1, 512], _F32, tag="r", name="r_s")
                    nc.vector.reciprocal_approx_fast(out=r_s, in_=rr)
                    rb = sm_p.tile([64, 512], _F32, tag="rb", name="rb")
                    nc.gpsimd.partition_broadcast(rb, r_s)
                    nc.vector.tensor_mul(
                        ctx_s[hp][hr:hr + 64, hf * 512:hf * 512 + 512],
                        ctx_ps[0:64, :], rb)
                pend.append(norm)
                drain(LAG)

            def emit_proj(q0, width):
                for dt in range(NDT):
                    o_ps = ps.tile([128, 1024], _F32, tag="ps2", name="o_ps")
                    for cc in range(0, width, 512):
                        for hp2 in range(2):
                            nc.tensor.matmul(
                                o_ps[:, cc:cc + 512],
                                lhsT=wp_s[hp2][:, dt * 128:(dt + 1) * 128],
                                rhs=ctx_s[hp2][:, q0 + cc:q0 + cc + 512],
                                start=(hp2 == 0), stop=(hp2 == 1),
                            )
                    o_s = ot_p.tile([128, 1024], _BF, tag="ot", name="o_s")
                    if dt % 2 == 0:
                        nc.scalar.copy(out=o_s[:, 0:width], in_=o_ps[:, 0:width])
                    else:
                        nc.vector.tensor_copy(out=o_s[:, 0:width],
                                              in_=o_ps[:, 0:width])
                    nc.sync.dma_start(
                        out=out[dt * 128:(dt + 1) * 128, q0:q0 + width],
                        in_=o_s[:, 0:width])

            # ================= schedule =================
            emit_A(0)
            emit_hf(0, 0)       # only needs window-0 data
            emit_hf(1, 0)
            emit_A(1)
            emit_hf(0, 1)
            emit_hf(1, 1)
            emit_hf(2, 0)
            emit_hf(2, 1)
            emit_hf(3, 0)
            emit_hf(3, 1)
            emit_hf(3, 2)
            emit_hf(2, 2)
            emit_proj(0, 1024)
            emit_hf(1, 2)
            emit_hf(0, 2)
            emit_hf(3, 3)
            emit_proj(1024, 512)
            emit_hf(2, 3)
            emit_hf(1, 3)
            emit_hf(0, 3)
            drain(0)
            emit_proj(1536, 512)
    nc.finalize()
    return nc


_NC = None


def _get_nc():
    global _NC
    if _NC is None:
        _NC = build_bass()
    return _NC


def _host_inputs(inputs, mask, Wqkv, bqkv, Wproj, bproj):
    x = np.asarray(inputs, np.float32)
    mask = np.asarray(mask)
    Wqkv = np.asarray(Wqkv, np.float32)
    bqkv = np.asarray(bqkv, np.float32)
    Wproj = np.asarray(Wproj, np.float32)

    start = 2.0 ** (-8.0 / H)
    slopes = start ** np.arange(1, H + 1, dtype=np.float64)

    per_g = {}
    ii = np.arange(128, dtype=np.float64)[:, None]
    jj = np.arange(512, dtype=np.float64)[None, :]
    for g in range(4):
        heads = [g, g + 4, g + 8, g + 12]
        wqk_ = np.empty((D, 2 * HPC * DH), np.float32)
        qkb_ = np.empty((128, 4), np.float32)
        wv_ = np.empty((D, HPC * DH), np.float32)
        wp_ = np.empty((HPC * DH, D), np.float32)
        etab_ = np.zeros((128, ETOT * 512), BF16)
        utab_ = np.empty((2 * S, 1), np.float32)
        for hl, hh in enumerate(heads):
            r0 = hh * 3 * DH
            p, half = hl // 2, hl % 2
            qcol = (2 * p) * 128 + half * 64
            kcol = (2 * p + 1) * 128 + half * 64
            wqk_[:, qcol:qcol + 64] = Wqkv[r0:r0 + DH, :].T * SCALE
            wqk_[:, kcol:kcol + 64] = Wqkv[r0 + DH:r0 + 2 * DH, :].T
            qkb_[half * 64:(half + 1) * 64, 2 * p] = bqkv[r0:r0 + DH] * SCALE
            qkb_[half * 64:(half + 1) * 64, 2 * p + 1] = bqkv[r0 + DH:r0 + 2 * DH]
            wv_[:, hl * 64:(hl + 1) * 64] = Wqkv[r0 + 2 * DH:r0 + 3 * DH, :].T
            wp_[hl * 64:(hl + 1) * 64, :] = Wproj[:, hh * DH:(hh + 1) * DH].T
            sl = slopes[hh]
            for dlt in EDELT[hl]:
                ei = EOFF[hl] + EIDX[hl][dlt]
                dd = dlt + ii - jj                    # k - q
                if hl < 2:
                    blk = np.exp(-sl * np.abs(dd))
                else:
                    blk = np.where(dd <= 0, 1.0, np.exp(-2.0 * sl * dd))
                etab_[:, ei * 512:(ei + 1) * 512] = blk
            if hl >= 2:
                kkk = np.arange(S, dtype=np.float64)
                utab_[(hl - 2) * S:(hl - 1) * S, 0] = np.exp(sl * (kkk - CENT))
        per_g[g] = dict(wqk=wqk_.astype(BF16), qkb=qkb_,
                        wv=wv_.astype(BF16),
                        wp=wp_.astype(BF16), etab=etab_, utab=utab_)

    in_maps = []
    for c in range(8):
        b, g = c // 4, c % 4
        m = dict(per_g[g])
        m["xt"] = np.ascontiguousarray(x[b].T).astype(BF16)
        m["mk"] = mask[b].astype(np.float32).reshape(S, 1)
        in_maps.append(m)
    return in_maps


def kernel(inputs, mask, Wqkv, bqkv, Wproj, bproj, _want_trace=False):
    nc = _get_nc()
    in_maps = _host_inputs(inputs, mask, Wqkv, bqkv, Wproj, bproj)
    res = run_bass_kernel_spmd(nc, in_maps, core_ids=list(range(8)),
                               trace=_want_trace)
    outs = res.results
    out = np.zeros((B, S, D), np.float32)
    for c in range(8):
        out[c // 4] += np.asarray(outs[c]["out"], np.float32).T
    # v-bias flows through softmax (weights sum to 1) into a constant:
    bv = np.asarray(bqkv, np.float32).reshape(3 * H, DH)[2::3].reshape(D)
    out += np.asarray(Wproj, np.float32) @ bv + np.asarray(bproj, np.float32)
    if _want_trace:
        kernel.last_result = res
    return out


# revision 34
# speedup vs baseline: 1.0603x; 1.0050x over previous
"""AltAttention (B=2,S=2048,D=1024,H=16, ALiBi + key-mask) on 8 TRN2 cores.

Sharding: core c = (b = c//4, head-group g = c%4 -> heads {g, g+4, g+8, g+12}).
Each core computes QKV for its 4 heads, attention, and a partial output
projection (row-split Wproj).  Host sums the 4 partials per batch and adds
bproj + Wproj @ bv (v-bias passes through softmax into a constant).

On-chip layout fully transposed: scores S^T=[k,q], context ctx^T=[dh,q],
output out^T=[dout,q].  All matmuls bf16 with fp32 PSUM.

v3 structure:
 - banding at (kt, half-window[512]) granularity, tiles with
   exp(-slope|k-q|) < e^-15 skipped; kt tiles processed in PAIRS sharing
   one [128,1024] PSUM tile and one ACT exp instruction.
 - heads 0,1 (steep): P = exp(S)*E (E table per diagonal offset).
   heads 2,3 (shallow): exp(-sl|k-q|) = [u(k)*v(q)]*R, v(q) cancels in
   softmax, u(k) folded into scaled V copies (vstR), R-table multiply only
   for tiles touching k>q.  E/R multiplies alternate DVE / GpSimd.
 - softmax: rowsums accumulated as a 65th PV output row;
   reciprocal_approx_fast + gpsimd partition_broadcast + one DVE multiply,
   per (head, half-window) right after its PV chain (fast ctx release).
 - q/k biases folded into PSUM evacuation (tensor_scalar add).
 - input x loaded over 4 DMA queues; phase-B tiles that only need window 0
   emitted between the two phase-A windows.
 - output partials in bf16, summed in fp32 on host.
"""

import sys

for _p in ("/opt/trn_rl_repo", "/opt/pypackages"):
    if _p not in sys.path:
        sys.path.insert(0, _p)

import numpy as np
import ml_dtypes

import concourse.bass as bass
from concourse import bacc
import concourse.mybir as mybir
import concourse.tile as tile
from concourse.bass_utils import run_bass_kernel_spmd

BF16 = ml_dtypes.bfloat16

B, S, D, H = 2, 2048, 1024, 16
DH = D // H
HPC = 4
SCALE = D ** -0.5
NKT = S // 128       # 16
NHF = S // 512       # 4 half-windows
NDT = D // 128       # 8
CENT = 1024

CUTS = [24, 96, 384, 99999]


def _band(hl, hf):
    cut = CUTS[hl]
    q0, q1 = hf * 512, hf * 512 + 512
    return [kt for kt in range(NKT)
            if kt * 128 < q1 + cut and (kt + 1) * 128 > q0 - cut]


BANDS = [[_band(hl, hf) for hf in range(NHF)] for hl in range(4)]


def _needs_e(hl, kt, hf):
    dlt = kt * 128 - hf * 512
    return hl < 2 or dlt > -128


EDELT = {}
for hl in range(4):
    ds = set()
    for hf in range(NHF):
        for kt in BANDS[hl][hf]:
            if _needs_e(hl, kt, hf):
                ds.add(kt * 128 - hf * 512)
    EDELT[hl] = sorted(ds)
EIDX = {hl: {d: i for i, d in enumerate(EDELT[hl])} for hl in range(4)}
ESLOT = [len(EDELT[hl]) for hl in range(4)]
EOFF = [0, ESLOT[0], ESLOT[0] + ESLOT[1], ESLOT[0] + ESLOT[1] + ESLOT[2]]
ETOT = sum(ESLOT)

_F32 = mybir.dt.float32
_BF = mybir.dt.bfloat16

Exp = mybir.ActivationFunctionType.Exp


def build_bass():
    nc = bacc.Bacc(None, target_bir_lowering=False)
    xt = nc.declare_dram_parameter("xt", [D, S], _BF, isOutput=False)
    wqk = nc.declare_dram_parameter("wqk", [D, 2 * HPC * DH], _BF, isOutput=False)
    qkb = nc.declare_dram_parameter("qkb", [128, 4], _F32, isOutput=False)
    wv = nc.declare_dram_parameter("wv", [D, HPC * DH], _BF, isOutput=False)
    wp = nc.declare_dram_parameter("wp", [HPC * DH, D], _BF, isOutput=False)
    etab = nc.declare_dram_parameter("etab", [128, ETOT * 512], _BF, isOutput=False)
    utab = nc.declare_dram_parameter("utab", [2 * S, 1], _F32, isOutput=False)
    mk = nc.declare_dram_parameter("mk", [S, 1], _F32, isOutput=False)
    out = nc.declare_dram_parameter("out", [D, S], _BF, isOutput=True)

    with tile.TileContext(nc) as tc:
        with (
            tc.tile_pool(name="consts", bufs=1) as consts,
            tc.tile_pool(name="wqk_p", bufs=1) as wqk_p,
            tc.tile_pool(name="wv_p", bufs=1) as wv_p,
            tc.tile_pool(name="kqt_p", bufs=1) as kqt_p,
            tc.tile_pool(name="vst_p", bufs=1) as vst_p,
            tc.tile_pool(name="xt_p", bufs=16) as xt_p,
            tc.tile_pool(name="ear_p", bufs=1) as ear_p,
            tc.tile_pool(name="p_p", bufs=6) as p_p,
            tc.tile_pool(name="wp_p", bufs=1) as wp_p,
            tc.tile_pool(name="ot_p", bufs=2) as ot_p,
            tc.tile_pool(name="sm_p", bufs=3) as sm_p,
            tc.tile_pool(name="ps", bufs=3, space="PSUM") as ps,
            tc.tile_pool(name="psc", bufs=2, space="PSUM") as psc,
        ):
            # ---- phase-A loads spread over 4 DMA queues ----
            xts_w = [[None] * NDT for _ in range(2)]

            def load_xt(stp, dt, eng, split=False):
                t = xt_p.tile([128, 1024], _BF, tag="xt", name=f"xt{stp}_{dt}")
                if split:
                    for cc in (0, 512):
                        eng.dma_start(
                            out=t[:, cc:cc + 512],
                            in_=xt[dt * 128:(dt + 1) * 128,
                                   stp * 1024 + cc:stp * 1024 + cc + 512])
                else:
                    eng.dma_start(
                        out=t, in_=xt[dt * 128:(dt + 1) * 128,
                                      stp * 1024:(stp + 1) * 1024])
                xts_w[stp][dt] = t

            wqk_s = [None] * NDT

            def load_wqk(dt, eng):
                t = wqk_p.tile([128, 512], _BF, tag=f"wqk{dt}", name=f"wqk{dt}")
                eng.dma_start(out=t, in_=wqk[dt * 128:(dt + 1) * 128, :])
                wqk_s[dt] = t

            xt0_tiles = {}

            def load_xt0_half(dt, cc, eng):
                if dt not in xt0_tiles:
                    xt0_tiles[dt] = xt_p.tile([128, 1024], _BF, tag="xt",
                                              name=f"xt0_{dt}")
                    xts_w[0][dt] = xt0_tiles[dt]
                eng.dma_start(out=xt0_tiles[dt][:, cc:cc + 512],
                              in_=xt[dt * 128:(dt + 1) * 128, cc:cc + 512])

            for dt in (0, 1, 2, 3):
                load_wqk(dt, nc.scalar)
            load_wqk(4, nc.gpsimd)
            load_wqk(5, nc.gpsimd)
            for dt in (0, 1, 2):
                load_xt0_half(dt, 0, nc.sync)
            for dt in (3, 4, 5):
                load_xt0_half(dt, 0, nc.scalar)
            load_wqk(6, nc.gpsimd)
            load_xt0_half(6, 0, nc.gpsimd)
            load_wqk(7, nc.gpsimd)
            load_xt0_half(7, 0, nc.gpsimd)
            for dt in (0, 1, 2):
                load_xt0_half(dt, 512, nc.sync)
            for dt in (3, 4, 5):
                load_xt0_half(dt, 512, nc.scalar)
            for dt in (6, 7):
                load_xt0_half(dt, 512, nc.gpsimd)
            wv_s = []
            for dt in range(NDT):
                t = wv_p.tile([128, 256], _BF, tag=f"wv{dt}", name=f"wv{dt}")
                nc.gpsimd.dma_start(out=t, in_=wv[dt * 128:(dt + 1) * 128, :])
                wv_s.append(t)
            for dt in (0, 1, 2):
                load_xt(1, dt, nc.sync)
            for dt in (3, 4, 5):
                load_xt(1, dt, nc.scalar)
            for dt in (6, 7):
                load_xt(1, dt, nc.gpsimd)

            # ---- ACT exp table warm-up ----
            dum = consts.tile([1, 1], _F32)
            nc.vector.memset(dum, 0.0)
            nc.scalar.activation(dum, dum, Exp)

            qkb_s = consts.tile([128, 4], _F32)
            nc.gpsimd.dma_start(out=qkb_s, in_=qkb[:, :])
            mk_s = consts.tile([128, NKT], _F32)
            nc.gpsimd.dma_start(
                out=mk_s, in_=mk.rearrange("(f p) a -> p (f a)", p=128))
            utab_s = consts.tile([128, 2 * NKT], _F32)
            nc.gpsimd.dma_start(
                out=utab_s, in_=utab.rearrange("(j f p) a -> p (j f a)",
                                               j=2, p=128))
            wp_s = []
            for hp in range(2):
                t = wp_p.tile([128, D], _BF, tag=f"wp{hp}", name=f"wp{hp}")
                nc.scalar.dma_start(out=t, in_=wp[hp * 128:(hp + 1) * 128, :])
                wp_s.append(t)
            # E tables: h0+h1 slots first (needed early), rest behind
            earena = ear_p.tile([128, ETOT * 512], _BF)
            c01 = EOFF[2] * 512  # columns for heads 0,1
            nc.sync.dma_start(out=earena[:, 0:c01], in_=etab[:, 0:c01])
            rest = ETOT * 512 - c01
            nch = 4
            w_ = rest // nch
            for c4 in range(nch):
                lo = c01 + c4 * w_
                hi = c01 + (c4 + 1) * w_ + (rest - nch * w_ if c4 == nch - 1 else 0)
                nc.sync.dma_start(out=earena[:, lo:hi], in_=etab[:, lo:hi])

            # ---- persistent activation tensors ----
            qq = [kqt_p.tile([128, S], _BF, tag=f"qq{p}", name=f"qq{p}")
                  for p in range(2)]
            kk = [kqt_p.tile([128, S], _BF, tag=f"kk{p}", name=f"kk{p}")
                  for p in range(2)]
            vst = [vst_p.tile([128, HPC * 65], _BF, tag=f"vst{kt}", name=f"vst{kt}")
                   for kt in range(NKT)]
            vstR = [vst_p.tile([128, 2 * 65], _BF, tag=f"vstR{kt}", name=f"vstR{kt}")
                    for kt in range(NKT)]
            ctx_s = [kqt_p.tile([128, S], _BF, tag=f"cs{hp}", name=f"cs{hp}")
                     for hp in range(2)]

            for kt in range(NKT):
                for h in range(HPC):
                    nc.vector.memset(vst[kt][:, h * 65 + 64:h * 65 + 65], 1.0)

            # ================= phase A (one window) =================
            def emit_A(stp):
                xts = xts_w[stp]
                for rt in range(HPC):
                    qk_ps = ps.tile([128, 1024], _F32, tag="ps2", name="qk_ps")
                    for c0 in (0, 512):
                        for dt in range(NDT):
                            nc.tensor.matmul(
                                qk_ps[:, c0:c0 + 512],
                                lhsT=wqk_s[dt][:, rt * 128:(rt + 1) * 128],
                                rhs=xts[dt][:, c0:c0 + 512],
                                start=(dt == 0), stop=(dt == NDT - 1),
                            )
                    dst = (qq if rt % 2 == 0 else kk)[rt // 2]
                    nc.vector.tensor_scalar_add(
                        dst[:, stp * 1024:(stp + 1) * 1024], qk_ps,
                        qkb_s[:, rt:rt + 1])
                for sg in range(2):
                    v_ps = ps.tile([128, 1024], _F32, tag="ps2", name="v_ps")
                    for s4 in range(4):
                        sub = sg * 4 + s4
                        for dt in range(NDT):
                            nc.tensor.matmul(
                                v_ps[:, s4 * 256:s4 * 256 + 256],
                                lhsT=xts[dt][:, sub * 128:(sub + 1) * 128],
                                rhs=wv_s[dt],
                                start=(dt == 0), stop=(dt == NDT - 1),
                            )
                    for s4 in range(4):
                        kt_i = stp * 8 + sg * 4 + s4
                        for h in range(HPC):
                            nc.vector.tensor_copy(
                                out=vst[kt_i][:, h * 65:h * 65 + 64],
                                in_=v_ps[:, s4 * 256 + h * 64:s4 * 256 + (h + 1) * 64])
                        nc.vector.tensor_scalar_mul(
                            vst[kt_i][:, :], vst[kt_i][:, :],
                            mk_s[:, kt_i:kt_i + 1])
                        for j in range(2):
                            nc.vector.tensor_scalar_mul(
                                vstR[kt_i][:, j * 65:(j + 1) * 65],
                                vst[kt_i][:, (2 + j) * 65:(3 + j) * 65],
                                utab_s[:, j * NKT + kt_i:j * NKT + kt_i + 1])

            # ================= phase B (software-pipelined) =================
            # tensor engine is strictly in-order: a PV matmul waiting on
            # exp+E of its own tile would stall the next score matmul.  So
            # PV groups and norms are deferred through a small lag queue and
            # emitted behind the next pairs' score/exp fronts.
            emul_tog = [0]
            pend = []
            LAG = 4

            def drain(keep):
                while len(pend) > keep:
                    pend.pop(0)()

            def emit_hf(h, hf):
                """score+exp+E+PV and softmax-normalize (head h, half-window hf)."""
                hp, half = h // 2, h % 2
                lo, hi = half * 64, half * 64 + 64
                hr = half * 64
                kts = BANDS[h][hf]
                ctx_ps = psc.tile([65, 512], _F32, tag="ctx", name="ctx_ps")
                qs = qq[hp][lo:hi, hf * 512:hf * 512 + 512]
                pairs = [kts[i:i + 2] for i in range(0, len(kts), 2)]
                npairs = len(pairs)
                for pi, pr in enumerate(pairs):
                    s2 = ps.tile([128, 1024], _F32, tag="ps2", name="s2")
                    for j, kt in enumerate(pr):
                        nc.tensor.matmul(
                            s2[:, j * 512:j * 512 + 512],
                            lhsT=kk[hp][lo:hi, kt * 128:(kt + 1) * 128],
                            rhs=qs, start=True, stop=True,
                        )
                    wd = len(pr) * 512
                    p2 = p_p.tile([128, 1024], _BF, tag="p", name="p2")
                    nc.scalar.activation(p2[:, 0:wd], s2[:, 0:wd], Exp)
                    for j, kt in enumerate(pr):
                        if _needs_e(h, kt, hf):
                            ei = EOFF[h] + EIDX[h][kt * 128 - hf * 512]
                            nc.vector.tensor_mul(
                                p2[:, j * 512:j * 512 + 512],
                                p2[:, j * 512:j * 512 + 512],
                                earena[:, ei * 512:(ei + 1) * 512])

                    def pv_group(pr=pr, p2=p2, pi=pi):
                        for j, kt in enumerate(pr):
                            if h < 2:
                                lhsT = vst[kt][:, h * 65:(h + 1) * 65]
                            else:
                                lhsT = vstR[kt][:, (h - 2) * 65:(h - 1) * 65]
                            nc.tensor.matmul(
                                ctx_ps, lhsT=lhsT,
                                rhs=p2[:, j * 512:j * 512 + 512],
                                start=(pi == 0 and j == 0),
                                stop=(pi == npairs - 1 and j == len(pr) - 1),
                                skip_group_check=True,
                            )
                    pend.append(pv_group)
                    drain(LAG)

                def norm():
                    rr = sm_p.tile([1, 512], _F32, tag="rr", name="rr")
                    nc.scalar.copy(out=rr, in_=ctx_ps[64:65, :])
                    r_s = sm_p.tile([1, 512], _F32, tag="r", name="r_s")
                    nc.vector.reciprocal_approx_fast(out=r_s, in_=rr)
                    rb = sm_p.tile([64, 512], _F32, tag="rb", name="rb")
                    nc.gpsimd.partition_broadcast(rb, r_s)
                    nc.vector.tensor_mul(
                        ctx_s[hp][hr:hr + 64, hf * 512:hf * 512 + 512],
                        ctx_ps[0:64, :], rb)
                pend.append(norm)
                drain(LAG)

            def emit_proj(q0, width):
                for dt in range(NDT):
                    o_ps = ps.tile([128, 1024], _F32, tag="ps2", name="o_ps")
                    for cc in range(0, width, 512):
                        for hp2 in range(2):
                            nc.tensor.matmul(
                                o_ps[:, cc:cc + 512],
                                lhsT=wp_s[hp2][:, dt * 128:(dt + 1) * 128],
                                rhs=ctx_s[hp2][:, q0 + cc:q0 + cc + 512],
                                start=(hp2 == 0), stop=(hp2 == 1),
                            )
                    o_s = ot_p.tile([128, 1024], _BF, tag="ot", name="o_s")
                    if dt % 2 == 0:
                        nc.scalar.copy(out=o_s[:, 0:width], in_=o_ps[:, 0:width])
                    else:
                        nc.vector.tensor_copy(out=o_s[:, 0:width],
                                              in_=o_ps[:, 0:width])
                    nc.sync.dma_start(
                        out=out[dt * 128:(dt + 1) * 128, q0:q0 + width],
                        in_=o_s[:, 0:width])

            # ================= schedule =================
            emit_A(0)
            emit_hf(0, 0)       # only needs window-0 data
            emit_hf(1, 0)
            emit_A(1)
            emit_hf(0, 1)
            emit_hf(1, 1)
            emit_hf(2, 0)
            emit_hf(2, 1)
            emit_hf(3, 0)
            emit_hf(3, 1)
            emit_hf(3, 2)
            emit_hf(2, 2)
            emit_proj(0, 1024)
            emit_hf(1, 2)
            emit_hf(0, 2)
            emit_hf(3, 3)
            emit_proj(1024, 512)
            emit_hf(2, 3)
            emit_hf(1, 3)
            emit_hf(0, 3)
            drain(0)
            emit_proj(1536, 512)
    nc.finalize()
    return nc


_NC = None


def _get_nc():
    global _NC
    if _NC is None:
        _NC = build_bass()
    return _NC


def _host_inputs(inputs, mask, Wqkv, bqkv, Wproj, bproj):
    x = np.asarray(inputs, np.float32)
    mask = np.asarray(mask)
    Wqkv = np.asarray(Wqkv, np.float32)
    bqkv = np.asarray(bqkv, np.float32)
    Wproj = np.asarray(Wproj, np.float32)

    start = 2.0 ** (-8.0 / H)
    slopes = start ** np.arange(1, H + 1, dtype=np.float64)

    per_g = {}
    ii = np.arange(128, dtype=np.float64)[:, None]
    jj = np.arange(512, dtype=np.float64)[None, :]
    for g in range(4):
        heads = [g, g + 4, g + 8, g + 12]
        wqk_ = np.empty((D, 2 * HPC * DH), np.float32)
        qkb_ = np.empty((128, 4), np.float32)
        wv_ = np.empty((D, HPC * DH), np.float32)
        wp_ = np.empty((HPC * DH, D), np.float32)
        etab_ = np.zeros((128, ETOT * 512), BF16)
        utab_ = np.empty((2 * S, 1), np.float32)
        for hl, hh in enumerate(heads):
            r0 = hh * 3 * DH
            p, half = hl // 2, hl % 2
            qcol = (2 * p) * 128 + half * 64
            kcol = (2 * p + 1) * 128 + half * 64
            wqk_[:, qcol:qcol + 64] = Wqkv[r0:r0 + DH, :].T * SCALE
            wqk_[:, kcol:kcol + 64] = Wqkv[r0 + DH:r0 + 2 * DH, :].T
            qkb_[half * 64:(half + 1) * 64, 2 * p] = bqkv[r0:r0 + DH] * SCALE
            qkb_[half * 64:(half + 1) * 64, 2 * p + 1] = bqkv[r0 + DH:r0 + 2 * DH]
            wv_[:, hl * 64:(hl + 1) * 64] = Wqkv[r0 + 2 * DH:r0 + 3 * DH, :].T
            wp_[hl * 64:(hl + 1) * 64, :] = Wproj[:, hh * DH:(hh + 1) * DH].T
            sl = slopes[hh]
            for dlt in EDELT[hl]:
                ei = EOFF[hl] + EIDX[hl][dlt]
                dd = dlt + ii - jj                    # k - q
                if hl < 2:
                    blk = np.exp(-sl * np.abs(dd))
                else:
                    blk = np.where(dd <= 0, 1.0, np.exp(-2.0 * sl * dd))
                etab_[:, ei * 512:(ei + 1) * 512] = blk
            if hl >= 2:
                kkk = np.arange(S, dtype=np.float64)
                utab_[(hl - 2) * S:(hl - 1) * S, 0] = np.exp(sl * (kkk - CENT))
        per_g[g] = dict(wqk=wqk_.astype(BF16), qkb=qkb_,
                        wv=wv_.astype(BF16),
                        wp=wp_.astype(BF16), etab=etab_, utab=utab_)

    in_maps = []
    for c in range(8):
        b, g = c // 4, c % 4
        m = dict(per_g[g])
        m["xt"] = np.ascontiguousarray(x[b].T).astype(BF16)
        m["mk"] = mask[b].astype(np.float32).reshape(S, 1)
        in_maps.append(m)
    return in_maps


def kernel(inputs, mask, Wqkv, bqkv, Wproj, bproj, _want_trace=False):
    nc = _get_nc()
    in_maps = _host_inputs(inputs, mask, Wqkv, bqkv, Wproj, bproj)
    res = run_bass_kernel_spmd(nc, in_maps, core_ids=list(range(8)),
                               trace=_want_trace)
    outs = res.results
    out = np.zeros((B, S, D), np.float32)
    for c in range(8):
        out[c // 4] += np.asarray(outs[c]["out"], np.float32).T
    # v-bias flows through softmax (weights sum to 1) into a constant:
    bv = np.asarray(bqkv, np.float32).reshape(3 * H, DH)[2::3].reshape(D)
    out += np.asarray(Wproj, np.float32) @ bv + np.asarray(bproj, np.float32)
    if _want_trace:
        kernel.last_result = res
    return out
